# revision 25
# baseline (speedup 1.0000x reference)
"""CoAttention (LBSA) Trainium2 kernel.

Reference computation (per batch b):
    xw   = x @ W                       [T, D]
    eij  = tanh(xw @ x^T + b)          [T, T]   (b broadcast over last axis s)
    sc   = eij @ c                     [T]
    a    = softmax(where(mask, sc, -inf))       over T
    out  = a @ x                       [D]

Sharding: data-parallel over batch B=64 across 8 NeuronCores (8 batches/core).
weight/context/b replicated. Device works in the "transposed" domain:
    xwT[e,t]  = sum_d W[d,e] * xT[d,t]          (lhsT = W natural, rhs = xT)
    eijT[s,t] = sum_e xT[e,s] * xwT[e,t]        (lhsT = xT tile,  rhs = xwT)
    sc[t]     = sum_s c[s] * tanh(eijT[s,t] + b[s])
so the only transposed operand needed (x^T) is produced on the host once.
"""

import numpy as np

from concourse import bacc, tile, mybir
from concourse.bass_utils import run_bass_kernel_spmd

f32 = mybir.dt.float32
f32r = mybir.dt.float32r

B, T, D = 64, 512, 1024
NCORES = 8
BPC = B // NCORES          # batches per core
KT = D // 128              # 8 d/e partition tiles
ST = T // 128              # 4 s partition tiles

# dtype used for the two big matmul chains (and the scores reduction).
# float32r streams 1 row/cycle on the PE (vs 4 for float32) at slightly
# reduced multiply precision; flip to f32 if accuracy ever becomes an issue.
import os

MM_BIG = f32 if os.environ.get("COATT_MM", "f32r") == "f32" else f32r
MM_SCORES = MM_BIG

Tanh = mybir.ActivationFunctionType.Tanh
Exp = mybir.ActivationFunctionType.Exp
Alu = mybir.AluOpType
AxX = mybir.AxisListType.X


def _f32(ap):
    """View an MM_BIG-typed tile as plain float32 (no-op when MM_BIG is f32)."""
    return ap.bitcast(f32) if ap.dtype is not f32 else ap


def build_nc(stage=6):
    nc = bacc.Bacc(None)

    xt_d = nc.dram_tensor("xt", [BPC, D, T], f32, kind="ExternalInput")
    m_d = nc.dram_tensor("m01", [BPC, T], f32, kind="ExternalInput")
    w_d = nc.dram_tensor("w", [D, D], f32, kind="ExternalInput")
    c_d = nc.dram_tensor("cvec", [T], f32, kind="ExternalInput")
    b_d = nc.dram_tensor("bvec", [T], f32, kind="ExternalInput")
    id_d = nc.dram_tensor("ident", [128, 128], f32, kind="ExternalInput")
    on_d = nc.dram_tensor("ones", [1, 128], f32, kind="ExternalInput")

    out_d = nc.dram_tensor("out", [BPC, D], f32, kind="ExternalOutput")
    a_d = nc.dram_tensor("a", [BPC, T], f32, kind="ExternalOutput")

    with tile.TileContext(nc) as tc:
        with (
            tc.tile_pool(name="const", bufs=1) as cpool,
            tc.tile_pool(name="xt", bufs=2) as xpool,
            tc.tile_pool(name="xw", bufs=2) as xwpool,
            tc.tile_pool(name="th", bufs=2) as thpool,
            tc.tile_pool(name="small", bufs=2) as spool,
            tc.tile_pool(name="scr", bufs=2) as scpool,
            tc.tile_pool(name="pxw", bufs=2, space="PSUM") as pxw_pool,
            tc.tile_pool(name="pe", bufs=2, space="PSUM") as pe_pool,
            tc.tile_pool(name="psc", bufs=1, space="PSUM") as psc_pool,
            tc.tile_pool(name="pab", bufs=1, space="PSUM") as pab_pool,
            tc.tile_pool(name="ptr", bufs=1, space="PSUM") as ptr_pool,
        ):
            w_sb = cpool.tile([128, KT * D], MM_BIG)
            nc.sync.dma_start(
                out=w_sb[:].rearrange("p (k e) -> p k e", k=KT),
                in_=w_d.ap().bitcast(MM_BIG).rearrange("(k p) e -> p k e", p=128),
            )
            c_sb = cpool.tile([128, ST], MM_SCORES)
            nc.sync.dma_start(
                out=c_sb[:],
                in_=c_d.ap().bitcast(MM_SCORES).rearrange("(j p) -> p j", p=128),
            )
            b_sb = cpool.tile([128, ST], f32)
            nc.sync.dma_start(
                out=b_sb[:], in_=b_d.ap().rearrange("(j p) -> p j", p=128)
            )
            id_sb = cpool.tile([128, 128], f32)
            nc.sync.dma_start(out=id_sb[:], in_=id_d.ap())
            ones_sb = cpool.tile([1, 128], f32)
            nc.sync.dma_start(out=ones_sb[:], in_=on_d.ap())
            out_all = None
            if 5 <= stage < 40:
                out_all = cpool.tile([128, BPC * KT], f32, name="out_all")

            for i in range(BPC):
                xt_sb = xpool.tile([128, KT * T], MM_BIG)
                nc.sync.dma_start(
                    out=xt_sb[:].rearrange("p (k t) -> p k t", k=KT),
                    in_=xt_d.ap()[i].bitcast(MM_BIG).rearrange(
                        "(k p) t -> p k t", p=128
                    ),
                )
                m_sb = spool.tile([1, T], f32, tag="mrow")
                nc.sync.dma_start(out=m_sb[:], in_=m_d.ap()[i : i + 1, :])

                # ---- step 1: xwT[e, t] = sum_d W[d, e] xT[d, t] ----
                xw_sb = xwpool.tile([128, KT * T], MM_BIG)
                for e in range(KT):
                    pxw = pxw_pool.tile([128, T], f32)
                    for k in range(KT):
                        nc.tensor.matmul(
                            pxw[:],
                            lhsT=w_sb[:, k * D + e * 128 : k * D + (e + 1) * 128],
                            rhs=xt_sb[:, k * T : (k + 1) * T],
                            start=(k == 0),
                            stop=(k == KT - 1),
                        )
                    nc.vector.tensor_copy(xw_sb[:, e * T : (e + 1) * T], pxw[:])
                if stage <= 1:
                    continue

                # ---- step 2: eijT[s, t] = sum_e xT[e, s] xwT[e, t]; tanh ----
                th_sb = thpool.tile([128, ST * T], MM_SCORES)
                for st in range(ST):
                    pe_ = pe_pool.tile([128, T], f32)
                    for e in range(KT):
                        nc.tensor.matmul(
                            pe_[:],
                            lhsT=xt_sb[:, e * T + st * 128 : e * T + (st + 1) * 128],
                            rhs=xw_sb[:, e * T : (e + 1) * T],
                            start=(e == 0),
                            stop=(e == KT - 1),
                        )
                    nc.scalar.activation(
                        th_sb[:, st * T : (st + 1) * T],
                        pe_[:],
                        Tanh,
                        bias=b_sb[:, st : st + 1],
                    )

                if stage <= 2:
                    continue

                # ---- scores[t] = sum_s c[s] tanh(...)[s, t] ----
                psc = psc_pool.tile([1, T], f32)
                for st in range(ST):
                    nc.tensor.matmul(
                        psc[:],
                        lhsT=c_sb[:, st : st + 1],
                        rhs=th_sb[:, st * T : (st + 1) * T],
                        start=(st == 0),
                        stop=(st == ST - 1),
                    )

                if stage <= 3:
                    continue

                # ---- masked softmax over t (mask folded in after exp) ----
                mxneg = spool.tile([1, 1], f32, tag="mxneg")
                nc.vector.tensor_reduce(
                    mxneg[:], psc[:], axis=AxX, op=Alu.max, negate=True
                )
                if stage == 40:
                    continue
                e_sb = spool.tile([1, T], f32, tag="erow")
                nc.scalar.activation(e_sb[:], psc[:], Exp, bias=mxneg[:])
                if stage == 41:
                    continue
                au_sb = spool.tile([1, T], f32, tag="aurow")
                ssum = spool.tile([1, 1], f32, tag="ssum")
                nc.vector.tensor_mul(au_sb[:], e_sb[:], m_sb[:])
                nc.vector.reduce_sum(ssum[:], au_sb[:], axis=AxX)
                if stage == 42:
                    continue
                rcp = spool.tile([1, 1], f32, tag="rcp")
                nc.vector.reciprocal(rcp[:], ssum[:])
                a_sb = spool.tile([1, T], f32, tag="arow")
                nc.vector.tensor_scalar_mul(a_sb[:], au_sb[:], rcp[:])
                if stage == 43:
                    continue
                nc.sync.dma_start(out=a_d.ap()[i], in_=a_sb[:])

                if stage <= 4 or stage >= 40:
                    continue

                # ---- out[d] = sum_t xT[d, t] a[t] ----
                pab = pab_pool.tile([128, T], f32)
                nc.tensor.matmul(
                    pab[:], lhsT=ones_sb[:], rhs=a_sb[:], start=True, stop=True
                )
                a_bc = spool.tile([128, T], f32, tag="abc")
                nc.vector.tensor_copy(a_bc[:], pab[:])
                for k in range(KT):
                    scr = scpool.tile([128, T], f32)
                    nc.vector.tensor_mul(
                        scr[:], _f32(xt_sb[:, k * T : (k + 1) * T]), a_bc[:]
                    )
                    nc.vector.reduce_sum(
                        out_all[:, i * KT + k : i * KT + k + 1], scr[:], axis=AxX
                    )

            # ---- emit out rows: transpose [128, BPC*KT] -> [BPC*KT, 128] ----
            if 6 <= stage < 40:
                ptr = ptr_pool.tile([BPC * KT, 128], f32)
                nc.tensor.transpose(ptr[:], out_all[:], id_sb[:])
                orow = cpool.tile([BPC * KT, 128], f32)
                nc.vector.tensor_copy(orow[:], ptr[:])
                nc.sync.dma_start(
                    out=out_d.ap().rearrange("b (k d) -> (b k) d", k=KT),
                    in_=orow[:],
                )

    nc.finalize()
    return nc


_NC_CACHE = {}


def _get_nc():
    stage = int(os.environ.get("COATT_STAGE", "6"))
    if stage not in _NC_CACHE:
        _NC_CACHE[stage] = build_nc(stage)
    return _NC_CACHE[stage]


def make_in_maps(x, mask, weight, context_vector, b):
    xt = np.ascontiguousarray(x.transpose(0, 2, 1))          # [B, D, T]
    m01 = np.ascontiguousarray(mask.astype(np.float32))      # [B, T]
    cvec = np.ascontiguousarray(context_vector[:, 0].astype(np.float32))
    bvec = np.ascontiguousarray(b.astype(np.float32))
    w = np.ascontiguousarray(weight.astype(np.float32))
    ident = np.eye(128, dtype=np.float32)
    ones = np.ones((1, 128), dtype=np.float32)
    in_maps = []
    for c in range(NCORES):
        sl = slice(c * BPC, (c + 1) * BPC)
        in_maps.append(
            {
                "xt": xt[sl],
                "m01": m01[sl],
                "w": w,
                "cvec": cvec,
                "bvec": bvec,
                "ident": ident,
                "ones": ones,
            }
        )
    return in_maps


def kernel(x, mask, weight, context_vector, b):
    x = np.asarray(x, dtype=np.float32)
    mask = np.asarray(mask)
    weight = np.asarray(weight, dtype=np.float32)
    context_vector = np.asarray(context_vector, dtype=np.float32)
    b = np.asarray(b, dtype=np.float32)

    nc = _get_nc()
    in_maps = make_in_maps(x, mask, weight, context_vector, b)
    res = run_bass_kernel_spmd(nc, in_maps, core_ids=list(range(NCORES)))
    out = np.concatenate([r["out"] for r in res.results], axis=0)
    a = np.concatenate([r["a"] for r in res.results], axis=0)
    return out, a
